# revision 17
# baseline (speedup 1.0000x reference)
"""Trainium2 Bass kernel for nn_AttnNet: attention-pooling over sequence.

Reference computation (per batch b):
    act    = tanh(X @ W.T + b)          # [S, H]
    scores = act @ context              # [S]
    w      = exp(scores * mask)         # masked_fill(-1e-32) == *mask (exp(0)=1)
    out    = (X.T @ w) / sum(w)         # [H]

Sharding: pure data-parallel, 4 batches per core across 8 cores.

Device layout (per core), all X data in bf16:
    xt   [BPC, KC, 128, S]  bf16  xt[b,k,p,s] = X[b, s, 128k+p]  (X^T, h on partitions)
    xn   [BPC, S/128, 128, H] bf16  xn[b,c,p,h] = X[b, 128c+p, h] (X natural, s on partitions)
    wt   [KC, 128, H]       bf16  wt[k,p,o]   = W[o, 128k+p]     (W^T)
    bias [128, MC]          f32   bias[p,m]   = b[128m+p]
    ctx  [128, MC]          bf16  ctx[p,m]    = context[128m+p]
    mask [BPC, S]           f32
outputs:
    num  [BPC, 4, 512] f32  4 col-group partial pooled rows (host: sum axis=1, divide)
    den  [BPC, NSG]    f32  per-512-chunk partial softmax denominators (host: sum)

Pipeline per (batch, half=2048 seq; subgroups g0..g3 of 512):
    PE : act^T[o,s] psum = sum_k wt[k,m]^T @ xt[k]     (bf16, 16 MM per subgroup)
    ACT: act = tanh(psum + bias[m])                    (per-partition bias fusion)
    PE : scores col-tiled: 4 subgroups concurrently via tile_position=(0,32j)
    DVE: masked = scores * mask          ACT: w = exp(masked), accum_out -> den
    DMA: w row -> DRAM scratch -> read back as 4 columns [128,4]
    PE : pooling col-tiled: pool_ps[32cc] += w_col[cc]^T @ xn[chunk]  (M=1 MMs, x4 concurrent)
"""

import numpy as np
import ml_dtypes

import concourse.bass as bass
import concourse.tile as tile
from concourse import bacc, mybir
from concourse.bass_utils import run_bass_kernel_spmd

N_CORES = 8
B, S, H = 32, 4096, 512
BPC = B // N_CORES
P = 128
KC = H // P
MC = H // P
SG = 512
NSG = S // SG
NCH = S // P         # 32 s-chunks per batch (pooling granularity)
XT_TILE = 2048       # seq extent of one SBUF tile ("half")
NXT = S // XT_TILE
GPH = XT_TILE // SG  # subgroups per half = 4

F32 = mybir.dt.float32
BF16 = mybir.dt.bfloat16
BF = ml_dtypes.bfloat16

TRACE = False
LAST = {}


def build():
    nc = bacc.Bacc("TRN2", target_bir_lowering=False, num_devices=N_CORES)
    xt_d = nc.declare_dram_parameter("xt", [BPC, KC, P, S], BF16, isOutput=False)
    xn_d = nc.declare_dram_parameter("xn", [BPC, NXT, P, 4 * GPH, H], BF16, isOutput=False)
    wt_d = nc.declare_dram_parameter("wt", [KC, P, H], BF16, isOutput=False)
    bias_d = nc.declare_dram_parameter("bias", [P, MC], F32, isOutput=False)
    ctx_d = nc.declare_dram_parameter("ctx", [P, MC], BF16, isOutput=False)
    mask_d = nc.declare_dram_parameter("mask", [BPC, S], BF16, isOutput=False)
    num_d = nc.declare_dram_parameter("num", [BPC, 4, SG], F32, isOutput=True)
    den_d = nc.declare_dram_parameter("den", [BPC, NXT], F32, isOutput=True)

    Tanh = mybir.ActivationFunctionType.Tanh
    Exp = mybir.ActivationFunctionType.Exp

    with tile.TileContext(nc) as tc:
        with (
            tc.tile_pool(name="singles", bufs=1) as singles,
            tc.tile_pool(name="xtp", bufs=3) as xtp,
            tc.tile_pool(name="xnp", bufs=3) as xnp,
            tc.tile_pool(name="actpool", bufs=4) as actpool,
            tc.tile_pool(name="maskpool", bufs=2) as maskpool,
            tc.tile_pool(name="mskp", bufs=2) as mskp,
            tc.tile_pool(name="rows", bufs=3) as rows,
            tc.tile_pool(name="wcols", bufs=8) as wcols,
            tc.tile_pool(name="numr", bufs=4) as numr,
            tc.tile_pool(name="dens", bufs=2) as dens,
            tc.tile_pool(name="scratchd", bufs=6, space="DRAM") as scratchd,
            tc.tile_pool(name="actps", bufs=2, space="PSUM") as actps,
            tc.tile_pool(name="scps", bufs=2, space="PSUM") as scps,
            tc.tile_pool(name="poolps", bufs=2, space="PSUM") as poolps,
        ):
            wt_sb = singles.tile([P, KC, H], BF16)
            nc.sync.dma_start(out=wt_sb[:, :, :], in_=wt_d.ap().rearrange("k p h -> p k h"))
            ctx_sb = singles.tile([P, MC], BF16)
            nc.sync.dma_start(out=ctx_sb[:, :], in_=ctx_d.ap())
            bias_sb = singles.tile([P, MC], F32)
            nc.sync.dma_start(out=bias_sb[:, :], in_=bias_d.ap())

            pending_pool = None
            for b in range(BPC):
                mask_sb = maskpool.tile([1, S], BF16, tag="mask")
                nc.sync.dma_start(out=mask_sb[:, :], in_=mask_d.ap()[b : b + 1, :])
                den_sb = dens.tile([1, NXT], F32, tag="den")
                pool_ps = poolps.tile([P, SG], F32, tag="pool")

                for half in range(NXT):
                    xt_sb = xtp.tile([P, KC, XT_TILE], BF16, tag="xt")
                    for gl in range(GPH):
                        for k in range(KC):
                            s0 = half * XT_TILE + gl * SG
                            nc.sync.dma_start(
                                out=xt_sb[:, k, gl * SG : (gl + 1) * SG],
                                in_=xt_d.ap()[b, k, :, s0 : s0 + SG],
                            )
                    xn_sb = xnp.tile([P, 4 * GPH, SG], BF16, tag="xn")
                    nc.sync.dma_start(out=xn_sb[:, :, :], in_=xn_d.ap()[b, half])

                    pair_tiles = []
                    for pair in range(GPH // 2):
                        act_sb = actpool.tile([P, MC, 2 * SG], BF16, tag="act")
                        pair_tiles.append(act_sb)
                        for m in range(MC):
                            ps = actps.tile([P, 2 * SG], F32, tag="ps")
                            for sg in range(2):
                                ssl = slice((pair * 2 + sg) * SG, (pair * 2 + sg + 1) * SG)
                                for k in range(KC):
                                    nc.tensor.matmul(
                                        ps[:, sg * SG : (sg + 1) * SG],
                                        lhsT=wt_sb[:, k, m * P : (m + 1) * P],
                                        rhs=xt_sb[:, k, ssl],
                                        start=(k == 0),
                                        stop=(k == KC - 1),
                                        skip_group_check=True,
                                    )
                            nc.scalar.activation(
                                out=act_sb[:, m, :],
                                in_=ps[:, :],
                                func=Tanh,
                                bias=bias_sb[:, m : m + 1],
                            )

                    # scores for the 4 subgroups of this half, col-tiled
                    sps = scps.tile([P, SG], F32, tag="sps")
                    for m in range(MC):
                        for j in range(GPH):
                            nc.tensor.matmul(
                                sps[32 * j : 32 * j + 1, :],
                                lhsT=ctx_sb[:, m : m + 1],
                                rhs=pair_tiles[j // 2][:, m, (j % 2) * SG : (j % 2 + 1) * SG],
                                start=(m == 0),
                                stop=(m == MC - 1),
                                tile_position=(0, 32 * j),
                            )

                    msk = mskp.tile([1, XT_TILE], F32, tag="msk")
                    for gl in range(GPH):
                        g = half * GPH + gl
                        nc.vector.tensor_mul(
                            msk[:, gl * SG : (gl + 1) * SG],
                            sps[32 * gl : 32 * gl + 1, :],
                            mask_sb[:, g * SG : (g + 1) * SG],
                        )
                    w_row = rows.tile([1, XT_TILE], BF16, tag="w")
                    nc.scalar.activation(
                        out=w_row[:, :],
                        in_=msk[:, :],
                        func=Exp,
                        accum_out=den_sb[:, half : half + 1],
                    )
                    wsc = scratchd.tile([1, XT_TILE], BF16, tag="wsc")
                    nc.sync.dma_start(out=wsc[:, :], in_=w_row[:, :])
                    w_cols = wcols.tile([P, 4 * GPH], BF16, tag="wc")
                    nc.sync.dma_start(
                        out=w_cols[:, :],
                        in_=wsc[:, :].rearrange("a (c p) -> (a p) c", p=P),
                    )

                    def emit_pool(
                        pps=pool_ps, wcs=w_cols, xn=xn_sb, bb=b, hh=half
                    ):
                        for gl2 in range(GPH):
                            for cc in range(4):
                                ci = gl2 * 4 + cc
                                nc.tensor.matmul(
                                    pps[32 * cc : 32 * cc + 1, :],
                                    lhsT=wcs[:, ci : ci + 1],
                                    rhs=xn[:, ci, :],
                                    start=(hh == 0 and gl2 == 0),
                                    stop=(hh == NXT - 1 and gl2 == GPH - 1),
                                    tile_position=(0, 32 * cc),
                                    skip_group_check=True,
                                )
                        if hh == NXT - 1:
                            for j in range(4):
                                nr = numr.tile([1, SG], F32, tag="nr")
                                nc.vector.tensor_copy(
                                    nr[:, :], pps[32 * j : 32 * j + 1, :]
                                )
                                nc.sync.dma_start(
                                    out=num_d.ap()[bb, j : j + 1, :], in_=nr[:, :]
                                )

                    if pending_pool is not None:
                        pending_pool()
                    pending_pool = emit_pool

                nc.sync.dma_start(out=den_d.ap()[b : b + 1, :], in_=den_sb[:, :])

            if pending_pool is not None:
                pending_pool()

    nc.compile()
    return nc


_NC_CACHE = {}


def _get_nc():
    if "nc" not in _NC_CACHE:
        _NC_CACHE["nc"] = build()
    return _NC_CACHE["nc"]


def kernel(inputs, mask, W, b, context):
    X = np.asarray(inputs, dtype=np.float32)
    mask = np.asarray(mask)
    W = np.asarray(W, dtype=np.float32)
    b = np.asarray(b, dtype=np.float32)
    context = np.asarray(context, dtype=np.float32)

    nc = _get_nc()

    xt_full = np.ascontiguousarray(X.transpose(0, 2, 1)).reshape(B, KC, P, S).astype(BF)
    xn_full = np.ascontiguousarray(
        X.reshape(B, NXT, 4 * GPH, P, H).transpose(0, 1, 3, 2, 4)
    ).astype(BF)
    wt = np.ascontiguousarray(W.T).reshape(KC, P, H).astype(BF)
    bias_dev = np.ascontiguousarray(b.reshape(MC, P).T)
    ctx_dev = np.ascontiguousarray(context.reshape(MC, P).T).astype(BF)
    mask_f = mask.astype(BF)

    in_maps = []
    for c in range(N_CORES):
        in_maps.append(
            {
                "xt": xt_full[c * BPC : (c + 1) * BPC],
                "xn": xn_full[c * BPC : (c + 1) * BPC],
                "wt": wt,
                "bias": bias_dev,
                "ctx": ctx_dev,
                "mask": mask_f[c * BPC : (c + 1) * BPC],
            }
        )

    res = run_bass_kernel_spmd(nc, in_maps, core_ids=list(range(N_CORES)), trace=TRACE)
    LAST["exec_time_ns"] = res.exec_time_ns
    LAST["result"] = res

    out = np.empty((B, H), np.float32)
    for c in range(N_CORES):
        num = res.results[c]["num"].sum(axis=1)
        den = res.results[c]["den"].sum(axis=1)
        out[c * BPC : (c + 1) * BPC] = num / den[:, None]
    return out
